# revision 27
# baseline (speedup 1.0000x reference)
"""Multi-head attention kernel for Trainium2, head-parallel across 8 NeuronCores.

Math per head h (reference):
    scores  = X @ W[h] @ X.T / sqrt(D)          [N, N]
    weights = softmax(scores, axis=-1) + 1e-8
    out    += weights @ (X @ V[h])              [N, D], summed over heads

Sharding: H=40 heads split 5-per-core across 8 cores; X replicated.  Each core
computes the partial sum of its 5 heads' outputs; the host sums the 8 partials.

Per-core architecture ("pipe"): a software pipeline paced by the scalar (ACT)
engine, which exponentiates all 5*2048*2048 scores at 1 elem/lane/cycle and is
the roofline.  One window = one (head h, m-tile mt); transposed score layout
E[m, n] so both big contractions run over the partition axis:

    ACT: exp of window k  (4x [128,512] f32 PSUM -> quarters of an SBUF f16
         tile; 512 wide so every ACT read stays inside one PSUM bank, which
         measured ~3x faster on HW than multi-bank reads)
    PE : 4 score matmuls for window k+1   (into the slots ACT just freed;
         dedicated single-buffer pools scpA/scpB pin each score group to the
         matching exp's PSUM slot)
         4 XE matmuls for window k        (XE[d,n] += X[m,d]^T E[m,n], f32
         PSUM accumulate over mt, 4 banks)
    DVE: EACC += EXP(k)                   (f16 SBUF add, 2x mode)

The attention output uses the reassociation
    out_h = (E^T X V[h]) / d = XE^T V[h] / d,
so no per-(head,m-tile) XV tensor is needed: at each head end XE chunks are
evacuated to SBUF (f16), a single V^T @ XE matmul per chunk produces the
unnormalized head output, and softmax denominators run off the critical path
(GPSIMD partition_all_reduce -> DVE reciprocal -> DVE mul -> GPSIMD add).

All matmul operands are f16 (full PE rate); PSUM is fp32 (TRN2 requirement).
"""

import sys

import numpy as np

try:
    import concourse  # noqa: F401  (provided by the container's sitecustomize)
except ImportError:  # pragma: no cover
    for p in ("/opt/trn_rl_repo", "/root/.axon_site/_ro/trn_rl_repo"):
        if p not in sys.path:
            sys.path.insert(0, p)

N, D, H, NCORES = 2048, 128, 40, 8
HC = H // NCORES          # heads per core
NT = N // 128             # 128-row tiles of n/m
CH = N // 512             # 512-column chunks of n
SCALE = 1.0 / float(np.sqrt(np.float32(D)))

CFG = {"arch": "pipe", "exp_bufs": 4, "exp_width": 512}

_CACHE = {}


def _emit_pipe(ctx, tc, nc, X, W, V, out, cfg):
    from concourse import mybir, bass_isa
    from concourse.masks import make_identity

    f32 = mybir.dt.float32
    f16 = mybir.dt.float16
    Exp = mybir.ActivationFunctionType.Exp

    consts = ctx.enter_context(tc.tile_pool(name="consts", bufs=1))
    big = ctx.enter_context(tc.tile_pool(name="big", bufs=1))
    eaccp = ctx.enter_context(tc.tile_pool(name="eaccp", bufs=2))
    bcp = ctx.enter_context(tc.tile_pool(name="bcp", bufs=2))
    evacp = ctx.enter_context(tc.tile_pool(name="evacp", bufs=10))
    expp = ctx.enter_context(tc.tile_pool(name="expp", bufs=cfg["exp_bufs"]))
    smallp = ctx.enter_context(tc.tile_pool(name="smallp", bufs=3))

    idt = consts.tile([128, 128], f32, tag="idt")
    make_identity(nc, idt[:])
    ones = consts.tile([128, 128], f16, tag="ones")
    nc.gpsimd.memset(ones[:], 1.0)

    # ---- prologue ----
    # 6 bulk DMAs only: the per-DMA HWDGE issue cost (~0.6us) on the SP
    # queue would otherwise dominate the pipeline fill time.
    X_stage = big.tile([128, N], f32, tag="xstage")
    Xv = X.rearrange("(g t p) d -> g p t d", g=4, p=128)
    Xsv = X_stage.rearrange("p (g t d) -> g p t d", g=4, t=4)
    nc.sync.dma_start(out=Xsv[0], in_=Xv[0])
    Wf = big.tile([128, HC * 128], f32, tag="wf")
    Vf = big.tile([128, HC * 128], f32, tag="vf")
    nc.sync.dma_start(out=Wf.rearrange("d (h e) -> d h e", h=HC),
                      in_=W.rearrange("h d e -> d h e"))
    nc.sync.dma_start(out=Vf.rearrange("d (h e) -> d h e", h=HC),
                      in_=V.rearrange("h d e -> d h e"))
    for g in range(1, 4):
        nc.sync.dma_start(out=Xsv[g], in_=Xv[g])
    Wc = big.tile([128, HC * 128], f16, tag="wc")
    Vc = big.tile([128, HC * 128], f16, tag="vc")
    nc.vector.tensor_copy(Wc[:], Wf[:])
    nc.vector.tensor_copy(Vc[:], Vf[:])

    # X in natural [m, d] layout, f16 (lhsT of the XE matmuls), cast in
    # 512-col groups so it pipelines with the X DMAs
    Xc = big.tile([128, N], f16, tag="xc")
    for g in range(4):
        nc.vector.tensor_copy(Xc[:, g * 512:(g + 1) * 512],
                              X_stage[:, g * 512:(g + 1) * 512])

    # X^T and XWT[e, n] (all heads) through a scoped 4-slot PSUM pool that
    # is released before the steady-state pools open.  Allocation order is
    # arranged so that the banks the score pools will inherit (slots 0/1 ->
    # scpA/scpB) are last used by ACT-copied tiles: the score stream then
    # never waits on the DVE copy backlog (slots 2/3 -> XE banks only gate
    # the first XE matmuls).
    XT = big.tile([128, N], f16, tag="xt")
    XWT = big.tile([128, HC * N], f16, tag="xwt")
    with tc.tile_pool(name="pp", bufs=4, space="PSUM") as pp:
        for half in range(2):
            pt = pp.tile([128, 1024], f32, tag="pp", name="ptx")
            for j in range(8):
                nt = 8 * half + j
                nc.tensor.transpose(pt[:, j * 128:(j + 1) * 128],
                                    X_stage[:, nt * 128:(nt + 1) * 128],
                                    idt[:])
            dst = XT[:, half * 1024:(half + 1) * 1024]
            if half == 0:
                nc.scalar.copy(dst, pt[:])
            else:
                nc.vector.tensor_copy(dst, pt[:])
        for h in range(HC):
            for g in range(2):
                sct = pp.tile([128, 1024], f32, tag="pp", name="xwtt")
                for j in range(2):
                    c = 2 * g + j
                    nc.tensor.matmul(sct[:, j * 512:(j + 1) * 512],
                                     Wc[:, h * 128:(h + 1) * 128],
                                     XT[:, c * 512:(c + 1) * 512],
                                     start=True, stop=True)
                dst = XWT[:, h * N + g * 1024:h * N + (g + 1) * 1024]
                if h in (0, HC - 1):
                    nc.scalar.copy(dst, sct[:])
                else:
                    nc.vector.tensor_copy(dst, sct[:])

    # dedicated single-buffered score pools: group A (n cols 0:1024) and
    # group B (1024:2048) each pin to one 2-bank PSUM slot, so the next
    # window's A-scores wait exactly on this window's A-exp (never B's).
    scpA = ctx.enter_context(tc.tile_pool(name="scpA", bufs=1, space="PSUM"))
    scpB = ctx.enter_context(tc.tile_pool(name="scpB", bufs=1, space="PSUM"))
    avp = ctx.enter_context(tc.tile_pool(name="avp", bufs=4, space="PSUM"))

    OUT = big.tile([128, N], f32, tag="oacc")

    def emit_scores(h, mt, g, pool):
        sct = pool.tile([128, 1024], f32, tag="sct")
        for j in range(2):
            c = 2 * g + j
            nc.tensor.matmul(sct[:, j * 512:(j + 1) * 512],
                             XT[:, mt * 128:(mt + 1) * 128],
                             XWT[:, h * N + c * 512:h * N + (c + 1) * 512],
                             start=True, stop=True)
        return sct

    NW = HC * NT
    sc_pending = [emit_scores(0, 0, 0, scpA), emit_scores(0, 0, 1, scpB)]
    EACC = None
    XEs = None
    pending = None

    for k in range(NW):
        h, mt = divmod(k, NT)
        if mt == 0:
            EACC = eaccp.tile([128, N], f16, tag="eacc")
            XEs = [avp.tile([128, 512], f32, tag="av", name=f"xe{c}")
                   for c in range(CH)]

        # ACT: exponentiate this window's scores.  Flat multi-bank PSUM
        # reads measured ~3x slower on real HW than the cost model says, so
        # either read per-bank (exp_width=512) or hand the instruction an AP
        # pre-split at the bank boundary (exp_3d, half the instructions).
        e = expp.tile([128, N], f16, tag="exp")
        sa, sb = sc_pending
        if cfg.get("exp_3d"):
            # one instruction per score group, input pre-split at the PSUM
            # bank boundary via a 3D AP
            nc.scalar.activation(
                e[:, 0:1024].rearrange("p (b c) -> p b c", b=2),
                sa[:].rearrange("p (b c) -> p b c", b=2), Exp, scale=SCALE)
            nc.scalar.activation(
                e[:, 1024:2048].rearrange("p (b c) -> p b c", b=2),
                sb[:].rearrange("p (b c) -> p b c", b=2), Exp, scale=SCALE)
        else:
            ew = cfg.get("exp_width", 512)
            for o in range(0, 1024, ew):
                nc.scalar.activation(e[:, o:o + ew], sa[:, o:o + ew], Exp,
                                     scale=SCALE)
            for o in range(0, 1024, ew):
                nc.scalar.activation(e[:, 1024 + o:1024 + o + ew],
                                     sb[:, o:o + ew], Exp, scale=SCALE)

        # PE: next window's scores first (they must never sit behind a
        # blocked XE matmul), then this window's XE accumulation
        xsl = Xc[:, mt * 128:(mt + 1) * 128]
        if k + 1 < NW:
            nh, nmt = divmod(k + 1, NT)
            na = emit_scores(nh, nmt, 0, scpA)
            nb = emit_scores(nh, nmt, 1, scpB)
            sc_pending = [na, nb]
        for c in range(CH):
            nc.tensor.matmul(XEs[c][:], xsl, e[:, c * 512:(c + 1) * 512],
                             start=(mt == 0), stop=(mt == NT - 1))

        # DVE: accumulate exp row sums
        if mt == 0:
            nc.vector.tensor_copy(EACC[:], e[:])
        elif mt < NT - 1:
            nc.vector.tensor_add(EACC[:], EACC[:], e[:])

        if mt == 2 and pending is not None:
            # deferred softmax tail of the previous head: by now its
            # partition_all_reduce has finished, so the DVE reciprocal and
            # the Pool muls/adds run without idling either queue.
            ph, pBCfull, poutPsb = pending
            BCr = bcp.tile([128, N], f16, tag="bcr")
            with nc.allow_low_precision(reason="1/d in f16, d~2048"):
                nc.vector.reciprocal(BCr[:], pBCfull[:])
            for c in range(CH):
                ncol = slice(c * 512, (c + 1) * 512)
                if ph == 0:
                    nc.gpsimd.tensor_mul(OUT[:, ncol], poutPsb[c][:],
                                         BCr[:, ncol])
                else:
                    tmp = smallp.tile([128, 512], f16, tag="tmp")
                    with nc.allow_low_precision(reason="per-head partial"):
                        nc.gpsimd.tensor_mul(tmp[:], poutPsb[c][:],
                                             BCr[:, ncol])
                    nc.gpsimd.tensor_add(OUT[:, ncol], OUT[:, ncol], tmp[:])
            pending = None

        if mt == NT - 1:
            # evacuate XE banks immediately, then finish this head's row-sum
            # (needed for exp-tile recycling) before the outP evacuations
            XEsb = [evacp.tile([128, 512], f16, tag="evac", name=f"xesb{c}")
                    for c in range(CH)]
            if h < HC - 1:
                for c in range(CH):
                    nc.vector.tensor_copy(XEsb[c][:], XEs[c][:])
                nc.vector.tensor_add(EACC[:], EACC[:], e[:])
            else:
                # drain: row-sum first (it gates the BC matmuls), evacs on
                # Pool so the DVE can start the normalize chain sooner
                nc.vector.tensor_add(EACC[:], EACC[:], e[:])
                for c in range(CH):
                    nc.vector.tensor_copy(XEsb[c][:], XEs[c][:])
            # unnormalized head output: outP[e, n] = V[h]^T @ XE through the
            # freshly-freed XE banks
            outPs = []
            for c in range(CH):
                op = avp.tile([128, 512], f32, tag="av", name=f"op{c}")
                nc.tensor.matmul(op[:], Vc[:, h * 128:(h + 1) * 128],
                                 XEsb[c][:], start=True, stop=True)
                outPs.append(op)
            if h < HC - 1:
                # evacuate outP so the next head's XE banks free right away;
                # softmax denominator on GPSIMD; reciprocal + normalize
                # deferred into the next head's pipeline (window mt==2)
                outPsb = []
                for c in range(CH):
                    ob = evacp.tile([128, 512], f16, tag="evac",
                                    name=f"opsb{c}")
                    nc.vector.tensor_copy(ob[:], outPs[c][:])
                    outPsb.append(ob)
                BCfull = bcp.tile([128, N], f16, tag="bcf")
                nc.gpsimd.partition_all_reduce(BCfull[:], EACC[:], 128,
                                               bass_isa.ReduceOp.add)
                pending = (h, BCfull, outPsb)
            else:
                # last head: drain fast with a PE ones-matmul denominator
                # (no Pool round-trip), chunk-by-chunk normalize, transpose
                # and store with one DMA per 512-row chunk
                bcs = []
                for half, pool in ((0, scpA), (1, scpB)):
                    bt = pool.tile([128, 1024], f32, tag="sct", name="bc")
                    for j in range(2):
                        c = 2 * half + j
                        nc.tensor.matmul(bt[:, j * 512:(j + 1) * 512],
                                         ones[:],
                                         EACC[:, c * 512:(c + 1) * 512],
                                         start=True, stop=True)
                    bcs.append(bt)
                for c in range(CH):
                    ncol = slice(c * 512, (c + 1) * 512)
                    bslice = bcs[c // 2][:, (c % 2) * 512:(c % 2 + 1) * 512]
                    BCrd = smallp.tile([128, 512], f16, tag="tmp",
                                       name="bcrd")
                    with nc.allow_low_precision(reason="1/d in f16"):
                        nc.vector.reciprocal(BCrd[:], bslice)
                    tmp = smallp.tile([128, 512], f16, tag="tmp")
                    with nc.allow_low_precision(reason="per-head partial"):
                        nc.vector.tensor_mul(tmp[:], outPs[c][:], BCrd[:])
                    nc.vector.tensor_add(OUT[:, ncol], OUT[:, ncol], tmp[:])
                    pt = avp.tile([128, 512], f32, tag="av", name="pt2")
                    for j in range(4):
                        nt = 4 * c + j
                        nc.tensor.transpose(pt[:, j * 128:(j + 1) * 128],
                                            OUT[:, nt * 128:(nt + 1) * 128],
                                            idt[:])
                    OUTN = smallp.tile([128, 512], f32, tag="outn")
                    nc.vector.tensor_copy(OUTN[:], pt[:])
                    nc.sync.dma_start(
                        out=out[c * 512:(c + 1) * 512, :].rearrange(
                            "(t p) d -> p t d", t=4),
                        in_=OUTN.rearrange("p (t d) -> p t d", t=4))



def _emit(ctx, tc, nc, X, W, V, out, cfg):
    return _emit_pipe(ctx, tc, nc, X, W, V, out, cfg)


def build(num_devices=NCORES, cfg=None, reps=None):
    import concourse.bacc as bacc
    import concourse.tile as tile
    from concourse import mybir
    from contextlib import ExitStack

    cfg = dict(CFG, **(cfg or {}))
    nc = bacc.Bacc("TRN2", target_bir_lowering=False, debug=False,
                   num_devices=num_devices)
    f32 = mybir.dt.float32
    X = nc.dram_tensor("X", [N, D], f32, kind="ExternalInput").ap()
    W = nc.dram_tensor("W", [HC, D, D], f32, kind="ExternalInput").ap()
    V = nc.dram_tensor("V", [HC, D, D], f32, kind="ExternalInput").ap()
    out = nc.dram_tensor("out", [N, D], f32, kind="ExternalOutput").ap()
    with tile.TileContext(nc) as tc:
        with ExitStack() as ctx:
            if reps:
                # benchmark mode: run the body `reps` times on-device
                with tc.For_i(0, reps, 1):
                    _emit(ctx, tc, nc, X, W, V, out, cfg)
            else:
                _emit(ctx, tc, nc, X, W, V, out, cfg)
    nc.compile()
    return nc


def _get_nc():
    key = tuple(sorted(CFG.items()))
    if key not in _CACHE:
        _CACHE[key] = build()
    return _CACHE[key]


def kernel(X, W, V):
    from concourse.bass_utils import run_bass_kernel_spmd

    X = np.ascontiguousarray(np.asarray(X, dtype=np.float32))
    W = np.ascontiguousarray(np.asarray(W, dtype=np.float32))
    V = np.ascontiguousarray(np.asarray(V, dtype=np.float32))
    nc = _get_nc()
    in_maps = [
        {"X": X,
         "W": np.ascontiguousarray(W[c * HC:(c + 1) * HC]),
         "V": np.ascontiguousarray(V[c * HC:(c + 1) * HC])}
        for c in range(NCORES)
    ]
    res = run_bass_kernel_spmd(nc, in_maps, list(range(NCORES)))
    partials = np.stack([res.results[c]["out"] for c in range(NCORES)])
    return partials.sum(axis=0, dtype=np.float32)


# revision 28
# speedup vs baseline: 1.3262x; 1.3262x over previous
"""Multi-head attention kernel for Trainium2, head-parallel across 8 NeuronCores.

Math per head h (reference):
    scores  = X @ W[h] @ X.T / sqrt(D)          [N, N]
    weights = softmax(scores, axis=-1) + 1e-8
    out    += weights @ (X @ V[h])              [N, D], summed over heads

Sharding: H=40 heads split 5-per-core across 8 cores; X replicated.  Each core
computes the partial sum of its 5 heads' outputs; the host sums the 8 partials.

Per-core architecture ("pipe"): a software pipeline paced by the scalar (ACT)
engine, which exponentiates all 5*2048*2048 scores at 1 elem/lane/cycle and is
the roofline.  One window = one (head h, m-tile mt); transposed score layout
E[m, n] so both big contractions run over the partition axis:

    ACT: exp of window k  (2 instructions of [128, 2, 512]: the access
         pattern is pre-split at the PSUM bank boundary, which avoids the
         ~3x multi-bank ACT read penalty measured on HW while paying the
         per-instruction overhead only twice per window)
    PE : 4 score matmuls for window k+1   (into the slots ACT just freed;
         dedicated single-buffer pools scpA/scpB pin each score group to the
         matching exp's PSUM slot)
         4 XE matmuls for window k        (XE[d,n] += X[m,d]^T E[m,n], f32
         PSUM accumulate over mt, 4 banks)
    DVE: EACC += EXP(k)                   (f16 SBUF add, 2x mode)

The attention output uses the reassociation
    out_h = (E^T X V[h]) / d = XE^T V[h] / d,
so no per-(head,m-tile) XV tensor is needed: at each head end XE chunks are
evacuated to SBUF (f16), a single V^T @ XE matmul per chunk produces the
unnormalized head output, and softmax denominators run off the critical path
(GPSIMD partition_all_reduce -> DVE reciprocal -> DVE mul -> GPSIMD add).

All matmul operands are f16 (full PE rate); PSUM is fp32 (TRN2 requirement).
"""

import sys

import numpy as np

try:
    import concourse  # noqa: F401  (provided by the container's sitecustomize)
except ImportError:  # pragma: no cover
    for p in ("/opt/trn_rl_repo", "/root/.axon_site/_ro/trn_rl_repo"):
        if p not in sys.path:
            sys.path.insert(0, p)

N, D, H, NCORES = 2048, 128, 40, 8
HC = H // NCORES          # heads per core
NT = N // 128             # 128-row tiles of n/m
CH = N // 512             # 512-column chunks of n
SCALE = 1.0 / float(np.sqrt(np.float32(D)))

CFG = {"arch": "pipe", "exp_bufs": 4, "exp_3d": True}

_CACHE = {}


def _emit_pipe(ctx, tc, nc, X, W, V, out, cfg):
    from concourse import mybir, bass_isa
    from concourse.masks import make_identity

    f32 = mybir.dt.float32
    f16 = mybir.dt.float16
    Exp = mybir.ActivationFunctionType.Exp

    consts = ctx.enter_context(tc.tile_pool(name="consts", bufs=1))
    big = ctx.enter_context(tc.tile_pool(name="big", bufs=1))
    eaccp = ctx.enter_context(tc.tile_pool(name="eaccp", bufs=2))
    bcp = ctx.enter_context(tc.tile_pool(name="bcp", bufs=2))
    evacp = ctx.enter_context(tc.tile_pool(name="evacp", bufs=10))
    expp = ctx.enter_context(tc.tile_pool(name="expp", bufs=cfg["exp_bufs"]))
    smallp = ctx.enter_context(tc.tile_pool(name="smallp", bufs=3))

    idt = consts.tile([128, 128], f32, tag="idt")
    make_identity(nc, idt[:])
    ones = consts.tile([128, 128], f16, tag="ones")
    nc.gpsimd.memset(ones[:], 1.0)

    # ---- prologue ----
    # 6 bulk DMAs only: the per-DMA HWDGE issue cost (~0.6us) on the SP
    # queue would otherwise dominate the pipeline fill time.
    X_stage = big.tile([128, N], f32, tag="xstage")
    Xv = X.rearrange("(g t p) d -> g p t d", g=4, p=128)
    Xsv = X_stage.rearrange("p (g t d) -> g p t d", g=4, t=4)
    nc.sync.dma_start(out=Xsv[0], in_=Xv[0])
    Wf = big.tile([128, HC * 128], f32, tag="wf")
    Vf = big.tile([128, HC * 128], f32, tag="vf")
    nc.sync.dma_start(out=Wf.rearrange("d (h e) -> d h e", h=HC),
                      in_=W.rearrange("h d e -> d h e"))
    nc.sync.dma_start(out=Vf.rearrange("d (h e) -> d h e", h=HC),
                      in_=V.rearrange("h d e -> d h e"))
    for g in range(1, 4):
        nc.sync.dma_start(out=Xsv[g], in_=Xv[g])
    Wc = big.tile([128, HC * 128], f16, tag="wc")
    Vc = big.tile([128, HC * 128], f16, tag="vc")
    nc.vector.tensor_copy(Wc[:], Wf[:])
    nc.vector.tensor_copy(Vc[:], Vf[:])

    # X in natural [m, d] layout, f16 (lhsT of the XE matmuls), cast in
    # 512-col groups so it pipelines with the X DMAs
    Xc = big.tile([128, N], f16, tag="xc")
    for g in range(4):
        nc.vector.tensor_copy(Xc[:, g * 512:(g + 1) * 512],
                              X_stage[:, g * 512:(g + 1) * 512])

    # X^T and XWT[e, n] (all heads) through a scoped 4-slot PSUM pool that
    # is released before the steady-state pools open.  Allocation order is
    # arranged so that the banks the score pools will inherit (slots 0/1 ->
    # scpA/scpB) are last used by ACT-copied tiles: the score stream then
    # never waits on the DVE copy backlog (slots 2/3 -> XE banks only gate
    # the first XE matmuls).
    XT = big.tile([128, N], f16, tag="xt")
    XWT = big.tile([128, HC * N], f16, tag="xwt")
    with tc.tile_pool(name="pp", bufs=4, space="PSUM") as pp:
        for half in range(2):
            pt = pp.tile([128, 1024], f32, tag="pp", name="ptx")
            for j in range(8):
                nt = 8 * half + j
                nc.tensor.transpose(pt[:, j * 128:(j + 1) * 128],
                                    X_stage[:, nt * 128:(nt + 1) * 128],
                                    idt[:])
            dst = XT[:, half * 1024:(half + 1) * 1024]
            if half == 0:
                nc.scalar.copy(dst, pt[:])
            else:
                nc.vector.tensor_copy(dst, pt[:])
        for h in range(HC):
            for g in range(2):
                sct = pp.tile([128, 1024], f32, tag="pp", name="xwtt")
                for j in range(2):
                    c = 2 * g + j
                    nc.tensor.matmul(sct[:, j * 512:(j + 1) * 512],
                                     Wc[:, h * 128:(h + 1) * 128],
                                     XT[:, c * 512:(c + 1) * 512],
                                     start=True, stop=True)
                dst = XWT[:, h * N + g * 1024:h * N + (g + 1) * 1024]
                if h in (0, HC - 1):
                    nc.scalar.copy(dst, sct[:])
                else:
                    nc.vector.tensor_copy(dst, sct[:])

    # dedicated single-buffered score pools: group A (n cols 0:1024) and
    # group B (1024:2048) each pin to one 2-bank PSUM slot, so the next
    # window's A-scores wait exactly on this window's A-exp (never B's).
    scpA = ctx.enter_context(tc.tile_pool(name="scpA", bufs=1, space="PSUM"))
    scpB = ctx.enter_context(tc.tile_pool(name="scpB", bufs=1, space="PSUM"))
    avp = ctx.enter_context(tc.tile_pool(name="avp", bufs=4, space="PSUM"))

    OUT = big.tile([128, N], f32, tag="oacc")

    def emit_scores(h, mt, g, pool):
        sct = pool.tile([128, 1024], f32, tag="sct")
        for j in range(2):
            c = 2 * g + j
            nc.tensor.matmul(sct[:, j * 512:(j + 1) * 512],
                             XT[:, mt * 128:(mt + 1) * 128],
                             XWT[:, h * N + c * 512:h * N + (c + 1) * 512],
                             start=True, stop=True)
        return sct

    NW = HC * NT
    sc_pending = [emit_scores(0, 0, 0, scpA), emit_scores(0, 0, 1, scpB)]
    EACC = None
    XEs = None
    pending = None

    for k in range(NW):
        h, mt = divmod(k, NT)
        if mt == 0:
            EACC = eaccp.tile([128, N], f16, tag="eacc")
            XEs = [avp.tile([128, 512], f32, tag="av", name=f"xe{c}")
                   for c in range(CH)]

        # ACT: exponentiate this window's scores.  Flat multi-bank PSUM
        # reads measured ~3x slower on real HW than the cost model says, so
        # either read per-bank (exp_width=512) or hand the instruction an AP
        # pre-split at the bank boundary (exp_3d, half the instructions).
        e = expp.tile([128, N], f16, tag="exp")
        sa, sb = sc_pending
        if cfg.get("exp_3d"):
            # one instruction per score group, input pre-split at the PSUM
            # bank boundary via a 3D AP
            nc.scalar.activation(
                e[:, 0:1024].rearrange("p (b c) -> p b c", b=2),
                sa[:].rearrange("p (b c) -> p b c", b=2), Exp, scale=SCALE)
            nc.scalar.activation(
                e[:, 1024:2048].rearrange("p (b c) -> p b c", b=2),
                sb[:].rearrange("p (b c) -> p b c", b=2), Exp, scale=SCALE)
        else:
            ew = cfg.get("exp_width", 512)
            for o in range(0, 1024, ew):
                nc.scalar.activation(e[:, o:o + ew], sa[:, o:o + ew], Exp,
                                     scale=SCALE)
            for o in range(0, 1024, ew):
                nc.scalar.activation(e[:, 1024 + o:1024 + o + ew],
                                     sb[:, o:o + ew], Exp, scale=SCALE)

        # PE: next window's scores first (they must never sit behind a
        # blocked XE matmul), then this window's XE accumulation
        xsl = Xc[:, mt * 128:(mt + 1) * 128]
        if k + 1 < NW:
            nh, nmt = divmod(k + 1, NT)
            na = emit_scores(nh, nmt, 0, scpA)
            nb = emit_scores(nh, nmt, 1, scpB)
            sc_pending = [na, nb]
        for c in range(CH):
            nc.tensor.matmul(XEs[c][:], xsl, e[:, c * 512:(c + 1) * 512],
                             start=(mt == 0), stop=(mt == NT - 1))

        # DVE: accumulate exp row sums
        if mt == 0:
            nc.vector.tensor_copy(EACC[:], e[:])
        elif mt < NT - 1:
            nc.vector.tensor_add(EACC[:], EACC[:], e[:])

        if mt == 2 and pending is not None:
            # deferred softmax tail of the previous head: by now its
            # partition_all_reduce has finished, so the DVE reciprocal and
            # the Pool muls/adds run without idling either queue.
            ph, pBCfull, poutPsb = pending
            BCr = bcp.tile([128, N], f16, tag="bcr")
            with nc.allow_low_precision(reason="1/d in f16, d~2048"):
                nc.vector.reciprocal(BCr[:], pBCfull[:])
            for c in range(CH):
                ncol = slice(c * 512, (c + 1) * 512)
                if ph == 0:
                    nc.gpsimd.tensor_mul(OUT[:, ncol], poutPsb[c][:],
                                         BCr[:, ncol])
                else:
                    tmp = smallp.tile([128, 512], f16, tag="tmp")
                    with nc.allow_low_precision(reason="per-head partial"):
                        nc.gpsimd.tensor_mul(tmp[:], poutPsb[c][:],
                                             BCr[:, ncol])
                    nc.gpsimd.tensor_add(OUT[:, ncol], OUT[:, ncol], tmp[:])
            pending = None

        if mt == NT - 1:
            # evacuate XE banks immediately, then finish this head's row-sum
            # (needed for exp-tile recycling) before the outP evacuations
            XEsb = [evacp.tile([128, 512], f16, tag="evac", name=f"xesb{c}")
                    for c in range(CH)]
            if h < HC - 1:
                for c in range(CH):
                    nc.vector.tensor_copy(XEsb[c][:], XEs[c][:])
                nc.vector.tensor_add(EACC[:], EACC[:], e[:])
            else:
                # drain: row-sum first (it gates the BC matmuls), evacs on
                # Pool so the DVE can start the normalize chain sooner
                nc.vector.tensor_add(EACC[:], EACC[:], e[:])
                for c in range(CH):
                    nc.vector.tensor_copy(XEsb[c][:], XEs[c][:])
            # unnormalized head output: outP[e, n] = V[h]^T @ XE through the
            # freshly-freed XE banks
            outPs = []
            for c in range(CH):
                op = avp.tile([128, 512], f32, tag="av", name=f"op{c}")
                nc.tensor.matmul(op[:], Vc[:, h * 128:(h + 1) * 128],
                                 XEsb[c][:], start=True, stop=True)
                outPs.append(op)
            if h < HC - 1:
                # evacuate outP so the next head's XE banks free right away;
                # softmax denominator on GPSIMD; reciprocal + normalize
                # deferred into the next head's pipeline (window mt==2)
                outPsb = []
                for c in range(CH):
                    ob = evacp.tile([128, 512], f16, tag="evac",
                                    name=f"opsb{c}")
                    nc.vector.tensor_copy(ob[:], outPs[c][:])
                    outPsb.append(ob)
                BCfull = bcp.tile([128, N], f16, tag="bcf")
                nc.gpsimd.partition_all_reduce(BCfull[:], EACC[:], 128,
                                               bass_isa.ReduceOp.add)
                pending = (h, BCfull, outPsb)
            else:
                # last head: drain fast with a PE ones-matmul denominator
                # (no Pool round-trip), chunk-by-chunk normalize, transpose
                # and store with one DMA per 512-row chunk
                bcs = []
                for half, pool in ((0, scpA), (1, scpB)):
                    bt = pool.tile([128, 1024], f32, tag="sct", name="bc")
                    for j in range(2):
                        c = 2 * half + j
                        nc.tensor.matmul(bt[:, j * 512:(j + 1) * 512],
                                         ones[:],
                                         EACC[:, c * 512:(c + 1) * 512],
                                         start=True, stop=True)
                    bcs.append(bt)
                for c in range(CH):
                    ncol = slice(c * 512, (c + 1) * 512)
                    bslice = bcs[c // 2][:, (c % 2) * 512:(c % 2 + 1) * 512]
                    BCrd = smallp.tile([128, 512], f16, tag="tmp",
                                       name="bcrd")
                    with nc.allow_low_precision(reason="1/d in f16"):
                        nc.vector.reciprocal(BCrd[:], bslice)
                    tmp = smallp.tile([128, 512], f16, tag="tmp")
                    with nc.allow_low_precision(reason="per-head partial"):
                        nc.vector.tensor_mul(tmp[:], outPs[c][:], BCrd[:])
                    nc.vector.tensor_add(OUT[:, ncol], OUT[:, ncol], tmp[:])
                    pt = avp.tile([128, 512], f32, tag="av", name="pt2")
                    for j in range(4):
                        nt = 4 * c + j
                        nc.tensor.transpose(pt[:, j * 128:(j + 1) * 128],
                                            OUT[:, nt * 128:(nt + 1) * 128],
                                            idt[:])
                    OUTN = smallp.tile([128, 512], f32, tag="outn")
                    nc.vector.tensor_copy(OUTN[:], pt[:])
                    nc.sync.dma_start(
                        out=out[c * 512:(c + 1) * 512, :].rearrange(
                            "(t p) d -> p t d", t=4),
                        in_=OUTN.rearrange("p (t d) -> p t d", t=4))



def _emit(ctx, tc, nc, X, W, V, out, cfg):
    return _emit_pipe(ctx, tc, nc, X, W, V, out, cfg)


def build(num_devices=NCORES, cfg=None, reps=None):
    import concourse.bacc as bacc
    import concourse.tile as tile
    from concourse import mybir
    from contextlib import ExitStack

    cfg = dict(CFG, **(cfg or {}))
    nc = bacc.Bacc("TRN2", target_bir_lowering=False, debug=False,
                   num_devices=num_devices)
    f32 = mybir.dt.float32
    X = nc.dram_tensor("X", [N, D], f32, kind="ExternalInput").ap()
    W = nc.dram_tensor("W", [HC, D, D], f32, kind="ExternalInput").ap()
    V = nc.dram_tensor("V", [HC, D, D], f32, kind="ExternalInput").ap()
    out = nc.dram_tensor("out", [N, D], f32, kind="ExternalOutput").ap()
    with tile.TileContext(nc) as tc:
        with ExitStack() as ctx:
            if reps:
                # benchmark mode: run the body `reps` times on-device
                with tc.For_i(0, reps, 1):
                    _emit(ctx, tc, nc, X, W, V, out, cfg)
            else:
                _emit(ctx, tc, nc, X, W, V, out, cfg)
    nc.compile()
    return nc


def _get_nc():
    key = tuple(sorted(CFG.items()))
    if key not in _CACHE:
        _CACHE[key] = build()
    return _CACHE[key]


def kernel(X, W, V):
    from concourse.bass_utils import run_bass_kernel_spmd

    X = np.ascontiguousarray(np.asarray(X, dtype=np.float32))
    W = np.ascontiguousarray(np.asarray(W, dtype=np.float32))
    V = np.ascontiguousarray(np.asarray(V, dtype=np.float32))
    nc = _get_nc()
    in_maps = [
        {"X": X,
         "W": np.ascontiguousarray(W[c * HC:(c + 1) * HC]),
         "V": np.ascontiguousarray(V[c * HC:(c + 1) * HC])}
        for c in range(NCORES)
    ]
    res = run_bass_kernel_spmd(nc, in_maps, list(range(NCORES)))
    partials = np.stack([res.results[c]["out"] for c in range(NCORES)])
    return partials.sum(axis=0, dtype=np.float32)
